# revision 68
# baseline (speedup 1.0000x reference)
"""Multi-head attention (B=2, S=2048, E=1024, H=16, causal) on 8 Trainium2 cores.

Sharding: data-parallel over batch (2) x tensor-parallel over heads (4 groups
of 4 heads). Core i handles batch i//4, heads 4*(i%4) .. 4*(i%4)+3.
Each core computes Q/K/V projections for its 256 channels, causal
flash-attention for its 4 heads, and a partial output projection
(contribution of its channels to all 1024 output features). Partials are
summed across the 4 cores of each batch group on the host (bo added there).

Schedule: PE must stay continuously busy (clock ramps to 2.4GHz only after
3us of uninterrupted execution) while ACT serially exps the score tiles.
A credit meter (emitted ACT-ns vs PE-ns) injects single projection /
out-proj matmul "filler" steps into the PE stream between each score and
its deferred PV, so the PE never waits on softmax.
"""
import numpy as np
import ml_dtypes

import concourse.bass as bass
import concourse.tile as tile
from concourse import bacc, mybir
from concourse.bass_utils import run_bass_kernel_spmd

F32 = mybir.dt.float32
F32R = mybir.dt.float32r
BF16 = mybir.dt.bfloat16
ActF = mybir.ActivationFunctionType
Alu = mybir.AluOpType

B, S, E = 2, 2048, 1024
H, DH = 16, 64
NCORES, TPW = 8, 4          # 8 cores, 4-way tensor parallel per batch
HPC = H // TPW              # heads per core = 4
C = HPC * DH                # channels per core = 256
SCALE = 1.0 / 8.0           # 1/sqrt(DH)
VW = HPC * (DH + 1)         # V storage width per s-tile (ones col per head)
NST = S // 128              # 16 s-tiles of 128 rows
NQB = S // 512              # 4 q-blocks of 512
NEC = E // 128              # 8 e-chunks (contraction for projections)

PE_NS = 1.0 / 2.4           # ns per PE cycle at full clock
ACT_NS = 1.0 / 1.2

_cache = {}


def _emit(nc, tc, causal):
    # ---- DRAM parameters ----
    xt_d = nc.dram_tensor("xt", [E, S], BF16, kind="ExternalInput").ap()
    wqt_d = nc.dram_tensor("wqt", [E, C], BF16, kind="ExternalInput").ap()
    wkt_d = nc.dram_tensor("wkt", [E, C], BF16, kind="ExternalInput").ap()
    wvt_d = nc.dram_tensor("wvt", [E, C], BF16, kind="ExternalInput").ap()
    wot_d = nc.dram_tensor("wot", [C, E], BF16, kind="ExternalInput").ap()
    bqk_d = nc.dram_tensor("bqk", [128, 4], F32, kind="ExternalInput").ap()
    bv_d = nc.dram_tensor("bv", [1, C], F32, kind="ExternalInput").ap()
    ones_d = nc.dram_tensor("ones", [1, 128], F32, kind="ExternalInput").ap()
    onesv_d = nc.dram_tensor("onesv", [128, NST * HPC], F32,
                             kind="ExternalInput").ap()
    out_d = nc.dram_tensor("out", [S, E], BF16, kind="ExternalOutput").ap()

    ctxpool = tc.tile_pool

    with ctxpool(name="persist", bufs=1) as pp:
        # ---- persistent SBUF tensors ----
        xt_sb = pp.tile([128, NEC * S], BF16)       # X^T, e-chunk ec at [ec*S)
        wqt_sb = pp.tile([128, NEC * C], BF16)
        wkt_sb = pp.tile([128, NEC * C], BF16)
        wvt_sb = pp.tile([128, NEC * C], BF16)
        wot_sb = pp.tile([128, 2 * E], BF16)        # c-chunk cc at [cc*E)
        qt_sb = pp.tile([128, 2 * S], BF16)         # Q^T, d-tile t at [t*S)
        kt_sb = pp.tile([128, 2 * S], BF16)
        v_sb = pp.tile([128, NST * VW], BF16)       # V (+ones col per head)
        ot_sb = pp.tile([128, 2 * S], BF16)         # normalized attn out^T
        bqk_sb = pp.tile([128, 4], F32)
        bvb_sb = pp.tile([128, C], F32)             # bv broadcast to partitions
        ones_r = pp.tile([1, 128], F32R)
        bv_row = pp.tile([1, C], F32R)
        onesb_sb = pp.tile([128, NST * HPC], F32)

        # ---- DMA: two queues in consumption order.
        # scalar queue: tiny tensors, wq, wv (frees up before first exp)
        # sync queue: x chunks + wk, then wot, then x for qb1..3 ----
        # Input stream in strict consumption order.  DIRECT2D bandwidth is a
        # shared ~200GB/s regardless of queue, so the head loads ONLY what
        # the first window needs: x-qb0 + the dt0 halves of wq/wk (1.5MB),
        # then wv, then the dt1 halves, then the rest.  Tiny tensors ride
        # the scalar queue interleaved so no whole-tensor record gates PE.
        # DMA bandwidth is ~200GB/s GLOBAL (shared by all queues), so strict
        # phase order matters: the 1.5MB the first window needs goes first
        # on every queue, then wv, then the dt1 weight halves, then the rest.
        smalls = [(bqk_sb, bqk_d, F32), (ones_r, ones_d, F32R),
                  (bv_row, bv_d, F32R), (onesb_sb, onesv_d, F32)]
        # phase 1: x-qb0 + dt0 halves of wq/wk
        for ec in range(NEC):
            nc.scalar.dma_start(
                out=wqt_sb[:, ec * C: ec * C + 128],
                in_=wqt_d[ec * 128:(ec + 1) * 128, 0:128])
            if smalls:
                dst, src, dt_ = smalls.pop(0)
                nc.scalar.dma_start(
                    out=dst[:], in_=src[:] if dt_ != F32R else src[:].bitcast(F32R))
            q = nc.sync if ec % 2 == 0 else nc.gpsimd
            q.dma_start(out=xt_sb[:, ec * S: ec * S + 512],
                        in_=xt_d[ec * 128:(ec + 1) * 128, 0:512])
            nc.sync.dma_start(
                out=wkt_sb[:, ec * C: ec * C + 128],
                in_=wkt_d[ec * 128:(ec + 1) * 128, 0:128])
        # phase 2+: wv then wq-dt1 on gpsimd (done before affines begin);
        # wk-dt1, wot, x-qb2/3 on sync; x-qb1 on scalar (frees by ~15us so
        # the exp stream is never queued behind a DMA)
        for ec in range(NEC):
            nc.gpsimd.dma_start(out=wvt_sb[:, ec * C:(ec + 1) * C],
                                in_=wvt_d[ec * 128:(ec + 1) * 128, :])
        for ec in range(NEC):
            nc.gpsimd.dma_start(
                out=wqt_sb[:, ec * C + 128: (ec + 1) * C],
                in_=wqt_d[ec * 128:(ec + 1) * 128, 128:C])
            nc.sync.dma_start(
                out=wkt_sb[:, ec * C + 128: (ec + 1) * C],
                in_=wkt_d[ec * 128:(ec + 1) * 128, 128:C])
            nc.scalar.dma_start(
                out=xt_sb[:, ec * S + 512: ec * S + 1024],
                in_=xt_d[ec * 128:(ec + 1) * 128, 512:1024])
        for cc in range(2):
            nc.sync.dma_start(out=wot_sb[:, cc * E:(cc + 1) * E],
                              in_=wot_d[cc * 128:(cc + 1) * 128, :])
        for ec in range(NEC):
            nc.sync.dma_start(
                out=xt_sb[:, ec * S + 1024: (ec + 1) * S],
                in_=xt_d[ec * 128:(ec + 1) * 128, 1024:S])

        # V ones columns via a strided DVE copy
        v_ones_ap = v_sb[:].rearrange("p (n x) -> p n x", x=DH + 1)[:, :, DH:DH + 1]
        nc.vector.tensor_copy(
            v_ones_ap, onesb_sb[:].rearrange("p (n x) -> p n x", x=1))

        with ctxpool(name="proj_ps", bufs=2, space="PSUM") as proj_ps, \
             ctxpool(name="score_ps", bufs=2, space="PSUM") as score_ps, \
             ctxpool(name="attn_ps", bufs=2, space="PSUM") as attn_ps, \
             ctxpool(name="pt_pool", bufs=4) as pt_pool, \
             ctxpool(name="rs_pool", bufs=4) as rs_pool, \
             ctxpool(name="bc_pool", bufs=4) as bc_pool, \
             ctxpool(name="out_pool", bufs=4) as out_pool:

            # ---------------- filler micro-steps ----------------
            def qkproj_steps(qb, dt, pj):
                w_sb = wqt_sb if pj == 0 else wkt_sb
                o_sb = qt_sb if pj == 0 else kt_sb
                st8 = {}

                def mk(e0):
                    def f():
                        if e0 == 0:
                            st8['ps'] = proj_ps.tile(
                                [128, 512], F32, tag="pp",
                                name=f"qk{qb}{dt}{pj}")
                        for ec in range(e0, e0 + 4):
                            nc.tensor.matmul(
                                st8['ps'][:],
                                w_sb[:, ec * C + dt * 128:
                                     ec * C + dt * 128 + 128],
                                xt_sb[:, ec * S + qb * 512:
                                      ec * S + qb * 512 + 512],
                                start=(ec == 0), stop=(ec == NEC - 1),
                                skip_group_check=True)
                        if e0 + 4 == NEC:
                            nc.vector.tensor_scalar_add(
                                o_sb[:, dt * S + qb * 512:
                                     dt * S + qb * 512 + 512],
                                st8['ps'][:],
                                bqk_sb[:, 2 * pj + dt: 2 * pj + dt + 1])
                    return (4 * 512 * PE_NS, f)
                return [mk(e0) for e0 in (0, 4)]

            def vproj_steps(st):
                st8 = {}

                def mk(e0):
                    def f():
                        if e0 == 0:
                            st8['ps'] = proj_ps.tile([128, C], F32, tag="pp",
                                                     name=f"vp{st}")
                        for ec in range(e0, e0 + 4):
                            nc.tensor.matmul(
                                st8['ps'][:],
                                xt_sb[:, ec * S + st * 128:
                                      ec * S + st * 128 + 128],
                                wvt_sb[:, ec * C: (ec + 1) * C],
                                start=(ec == 0), stop=(ec == NEC - 1),
                                skip_group_check=True)
                        if e0 + 4 == NEC:
                            dst = v_sb[:, st * VW: st * VW + VW].rearrange(
                                "p (h x) -> p h x", h=HPC)[:, :, 0:DH]
                            nc.vector.tensor_add(
                                dst,
                                st8['ps'][:].rearrange("p (h x) -> p h x",
                                                       h=HPC),
                                bvb_sb[:].rearrange("p (h x) -> p h x", h=HPC))
                    return (4 * 256 * PE_NS, f)
                return [mk(e0) for e0 in (0, 4)]

            def outproj_steps(st, tail=False):
                # atomic: the psum tile shares rotation with score tiles, so
                # all its writers+readers must be emitted contiguously
                def f():
                    ps = score_ps.tile([128, 1024], F32, tag="sc",
                                       name=f"op{st}")
                    for eb in range(2):
                        for cc in range(2):
                            nc.tensor.matmul(
                                ps[:, eb * 512:(eb + 1) * 512],
                                ot_sb[:, cc * S + st * 128:
                                      cc * S + st * 128 + 128],
                                wot_sb[:, cc * E + eb * 512:
                                       cc * E + eb * 512 + 512],
                                start=(cc == 0), stop=(cc == 1),
                                skip_group_check=True)
                    o_t = out_pool.tile([128, E], BF16, tag="ob",
                                        name=f"ot{st}")
                    nc.vector.tensor_copy(o_t[:, 0:512], ps[:, 0:512])
                    if tail:
                        # ACT/gpsimd are idle once the last exp retires;
                        # split work so the drain overlaps across queues
                        nc.scalar.copy(o_t[:, 512:1024], ps[:, 512:1024])
                        for h, q in ((0, nc.sync), (1, nc.gpsimd)):
                            q.dma_start(
                                out=out_d[st * 128:(st + 1) * 128,
                                          h * 512:(h + 1) * 512],
                                in_=o_t[:, h * 512:(h + 1) * 512])
                    else:
                        # mid-window: stay off the ACT/gpsimd queues (exp
                        # and affine_select must not be delayed)
                        nc.vector.tensor_copy(o_t[:, 512:1024],
                                              ps[:, 512:1024])
                        nc.sync.dma_start(
                            out=out_d[st * 128:(st + 1) * 128, :], in_=o_t[:])
                return [(4 * 512 * PE_NS, f)]

            # filler queue: (pe_ns, key, fn); key = win*100 + kt of first use
            fillers = []
            for qb in range(NQB):
                for dt in range(2):
                    if qb == 0 and dt == 0:
                        continue   # emitted directly in the head phase
                    for pj in range(2):
                        for s in qkproj_steps(qb, dt, pj):
                            fillers.append((s[0], (2 * qb + dt) * 100, s[1]))
            for st in range(0, NST):
                key = (2 * (st // 4) * 100 + st) if causal else st
                for s in vproj_steps(st):
                    fillers.append((s[0], key, s[1]))
            fillers.sort(key=lambda f: f[1])

            # per-window ACT / attention-PE ns and filler quotas (static)
            def w0_of(qb, kt):
                j = kt - 4 * qb
                return 128 * j if (causal and j > 0) else 0

            win_act, win_pe = [], []
            for qb in range(NQB):
                nk = 4 * (qb + 1) if causal else NST
                for hp in range(2):
                    a = p = 0.0
                    for kt in range(nk):
                        w = 512 - w0_of(qb, kt)
                        a += 2 * w * ACT_NS + 190
                        p += 3 * w * PE_NS
                    win_act.append(a)
                    win_pe.append(p)
            total_filler = sum(f[0] for f in fillers) + 16 * 4 * 512 * PE_NS
            deficits = [max(0.0, a - p) for a, p in zip(win_act, win_pe)]
            spare = max(0.0, total_filler - sum(deficits))
            quota = [d + spare / len(deficits) for d in deficits]

            mtr = {"popped": 0.0, "target": 0.0}

            def pop_fillers(force_key=None, pace=None):
                while fillers:
                    pe_ns, key, fn = fillers[0]
                    if force_key is not None and key <= force_key:
                        pass
                    elif pace is not None and mtr["popped"] < pace:
                        pass
                    else:
                        break
                    fillers.pop(0)
                    fn()
                    mtr["popped"] += pe_ns

            # ---------------- head phase: qb0/dt0 projections ----------------
            for pj in range(2):
                for _, f in qkproj_steps(0, 0, pj):
                    f()
            # bv broadcast to all partitions via K=1 matmul (deps are tiny
            # DMAs done long ago; placed here so it can't block the PE head)
            ps_bv = proj_ps.tile([128, C], F32, tag="pp", name="bvb")
            nc.tensor.matmul(ps_bv[:], ones_r[0:1, 0:128], bv_row[:],
                             start=True, stop=True)
            nc.vector.tensor_copy(bvb_sb[:], ps_bv[:])

            # ---------------- attention + out-proj ----------------
            pending = []

            def flush_pending():
                while pending:
                    pending.pop(0)()

            for qb in range(NQB):
                nk = 4 * (qb + 1) if causal else NST
                q0 = qb * 512
                for hp in range(2):
                    win = 2 * qb + hp
                    pop_fillers(force_key=win * 100)
                    win_base = mtr["popped"]
                    t = hp
                    ps_os = [None, None]

                    def emit_pv(kt, pt, w0, hp=hp, nk=nk, ps_os=ps_os, qb=qb):
                        if kt == 0:
                            for a in range(2):
                                ps_os[a] = attn_ps.tile(
                                    [65, 512], F32, tag="po",
                                    name=f"po{qb}{hp}{a}")
                        for a in range(2):
                            h = 2 * hp + a
                            nc.tensor.matmul(
                                ps_os[a][:, w0:512],
                                v_sb[:, kt * VW + h * (DH + 1):
                                     kt * VW + h * (DH + 1) + DH + 1],
                                pt[:, a * 512 + w0:(a + 1) * 512],
                                start=(kt == 0), stop=(kt == nk - 1),
                                skip_group_check=True)


                    pv_queue = []
                    for kt in range(nk):
                        j = kt - 4 * qb
                        w0 = 128 * j if (causal and j > 0) else 0
                        ps_s = score_ps.tile([128, 1024], F32, tag="sc",
                                             name=f"sc{qb}{hp}{kt}")
                        pt = pt_pool.tile([128, 1024], BF16, tag="pt",
                                          name=f"pt{qb}{hp}{kt}")
                        for a in range(2):
                            p0 = a * 64
                            nc.tensor.matmul(
                                ps_s[:, a * 512 + w0:(a + 1) * 512],
                                kt_sb[p0:p0 + 64,
                                      t * S + kt * 128: t * S + kt * 128 + 128],
                                qt_sb[p0:p0 + 64,
                                      t * S + q0 + w0: t * S + q0 + 512],
                                start=True, stop=True)
                        ps3 = ps_s[:].rearrange("p (u q) -> p u q", u=2)
                        pt3 = pt[:].rearrange("p (u q) -> p u q", u=2)
                        nc.scalar.activation(pt3[:, :, w0:512],
                                             ps3[:, :, w0:512],
                                             ActF.Exp, scale=SCALE)
                        if causal and 0 <= j:
                            nc.gpsimd.affine_select(
                                out=pt3[:, :, w0:w0 + 128],
                                in_=pt3[:, :, w0:w0 + 128],
                                compare_op=Alu.is_ge,
                                fill=0.0, base=0,
                                pattern=[[0, 2], [1, 128]],
                                channel_multiplier=-1)
                        if kt == 0:
                            flush_pending()
                        pop_fillers(
                            force_key=win * 100 + kt,
                            pace=win_base + quota[win] * (kt + 1) / nk)
                        pv_queue.append((kt, pt, w0))
                        if len(pv_queue) > 2:
                            emit_pv(*pv_queue.pop(0))
                    while pv_queue:
                        emit_pv(*pv_queue.pop(0))

                    def norm(qb=qb, hp=hp, t=t, q0=q0, ps_os=ps_os):
                        rs = [None, None]
                        ps_b = [None, None]
                        bc = [None, None]
                        for a in range(2):
                            rs[a] = rs_pool.tile([1, 512], F32R, tag="rs",
                                                 name=f"rs{qb}{hp}{a}")
                            nc.vector.tensor_copy(rs[a][:],
                                                  ps_os[a][64:65, :])
                        for a in range(2):
                            ps_b[a] = score_ps.tile([64, 512], F32, tag="sc",
                                                    name=f"pb{qb}{hp}{a}")
                            nc.tensor.matmul(ps_b[a][:], ones_r[0:1, 0:64],
                                             rs[a][:], start=True, stop=True)
                        for a in range(2):
                            bc[a] = bc_pool.tile([64, 512], F32, tag="bc",
                                                 name=f"bc{qb}{hp}{a}")
                            nc.vector.reciprocal_approx_fast(bc[a][:],
                                                             ps_b[a][:])
                        for a in range(2):
                            p0 = a * 64
                            nc.vector.tensor_mul(
                                ot_sb[p0:p0 + 64,
                                      t * S + q0: t * S + q0 + 512],
                                ps_os[a][0:64, :], bc[a][:])
                        if hp == 1:
                            tail = qb == NQB - 1
                            for i, st in enumerate(range(qb * 4, qb * 4 + 4)):
                                # spread-force into window 2qb+4 (tail for qb3)
                                key = 999999 if tail else \
                                    (2 * qb + 4) * 100 + 1 + 3 * i
                                for s in outproj_steps(st, tail=tail):
                                    fillers.append((s[0], key, s[1]))
                            fillers.sort(key=lambda f: f[1])
                    pending.append(norm)
            flush_pending()
            pop_fillers(force_key=999999)


def _build(causal):
    nc = bacc.Bacc("TRN2", target_bir_lowering=False, debug=False,
                   num_devices=NCORES)
    with tile.TileContext(nc) as tc:
        _emit(nc, tc, causal)
    nc.compile()
    return nc


def _shard_inputs(QKV, Wq, bq, Wk, bk, Wv, bv, Wo, bo):
    QKV = np.asarray(QKV, dtype=np.float32)
    Wq, Wk, Wv, Wo = (np.asarray(w, dtype=np.float32) for w in (Wq, Wk, Wv, Wo))
    bq, bk, bv = (np.asarray(b_, dtype=np.float32) for b_ in (bq, bk, bv))
    ones = np.ones((1, 128), dtype=np.float32)
    onesv = np.ones((128, NST * HPC), dtype=np.float32)
    bf = ml_dtypes.bfloat16
    in_maps = []
    for core in range(NCORES):
        b, g = divmod(core, TPW)
        cs = slice(g * C, (g + 1) * C)
        bqs, bks = bq[cs], bk[cs]
        bqk = np.stack([bqs[:128], bqs[128:], bks[:128], bks[128:]], axis=1)
        in_maps.append({
            "xt": np.ascontiguousarray(QKV[b].T).astype(bf),
            "wqt": np.ascontiguousarray(Wq[cs, :].T).astype(bf),
            "wkt": np.ascontiguousarray(Wk[cs, :].T).astype(bf),
            "wvt": np.ascontiguousarray(Wv[cs, :].T).astype(bf),
            "wot": np.ascontiguousarray(Wo[:, cs].T).astype(bf),
            "bqk": np.ascontiguousarray(bqk),
            "bv": bv[cs].reshape(1, C).copy(),
            "ones": ones,
            "onesv": onesv,
        })
    return in_maps


def kernel(QKV, Wq, bq, Wk, bk, Wv, bv, Wo, bo, is_causal):
    causal = bool(int(np.asarray(is_causal)))
    if causal not in _cache:
        _cache[causal] = _build(causal)
    nc = _cache[causal]
    in_maps = _shard_inputs(QKV, Wq, bq, Wk, bk, Wv, bv, Wo, bo)
    res = run_bass_kernel_spmd(nc, in_maps, core_ids=list(range(NCORES)))
    bo_f = np.asarray(bo, dtype=np.float32)
    out = np.empty((B, S, E), dtype=np.float32)
    for b in range(B):
        acc = res.results[TPW * b]["out"].astype(np.float32)
        for g in range(1, TPW):
            acc = acc + res.results[TPW * b + g]["out"].astype(np.float32)
        out[b] = acc + bo_f
    return out


# revision 69
# speedup vs baseline: 1.0049x; 1.0049x over previous
"""Multi-head attention (B=2, S=2048, E=1024, H=16, causal) on 8 Trainium2 cores.

Sharding: data-parallel over batch (2) x tensor-parallel over heads (4 groups
of 4 heads). Core i handles batch i//4, heads 4*(i%4) .. 4*(i%4)+3.
Each core computes Q/K/V projections for its 256 channels, causal
flash-attention for its 4 heads, and a partial output projection
(contribution of its channels to all 1024 output features). Partials are
summed across the 4 cores of each batch group on the host (bo added there).

Schedule: PE must stay continuously busy (clock ramps to 2.4GHz only after
3us of uninterrupted execution) while ACT serially exps the score tiles.
A credit meter (emitted ACT-ns vs PE-ns) injects single projection /
out-proj matmul "filler" steps into the PE stream between each score and
its deferred PV, so the PE never waits on softmax.
"""
import numpy as np
import ml_dtypes

import concourse.bass as bass
import concourse.tile as tile
from concourse import bacc, mybir
from concourse.bass_utils import run_bass_kernel_spmd

F32 = mybir.dt.float32
F32R = mybir.dt.float32r
BF16 = mybir.dt.bfloat16
ActF = mybir.ActivationFunctionType
Alu = mybir.AluOpType

B, S, E = 2, 2048, 1024
H, DH = 16, 64
NCORES, TPW = 8, 4          # 8 cores, 4-way tensor parallel per batch
HPC = H // TPW              # heads per core = 4
C = HPC * DH                # channels per core = 256
SCALE = 1.0 / 8.0           # 1/sqrt(DH)
VW = HPC * (DH + 1)         # V storage width per s-tile (ones col per head)
NST = S // 128              # 16 s-tiles of 128 rows
NQB = S // 512              # 4 q-blocks of 512
NEC = E // 128              # 8 e-chunks (contraction for projections)

PE_NS = 1.0 / 2.4           # ns per PE cycle at full clock
ACT_NS = 1.0 / 1.2

_cache = {}


def _emit(nc, tc, causal):
    # ---- DRAM parameters ----
    xt_d = nc.dram_tensor("xt", [E, S], BF16, kind="ExternalInput").ap()
    wqt_d = nc.dram_tensor("wqt", [E, C], BF16, kind="ExternalInput").ap()
    wkt_d = nc.dram_tensor("wkt", [E, C], BF16, kind="ExternalInput").ap()
    wvt_d = nc.dram_tensor("wvt", [E, C], BF16, kind="ExternalInput").ap()
    wot_d = nc.dram_tensor("wot", [C, E], BF16, kind="ExternalInput").ap()
    bqk_d = nc.dram_tensor("bqk", [128, 4], F32, kind="ExternalInput").ap()
    bv_d = nc.dram_tensor("bv", [1, C], F32, kind="ExternalInput").ap()
    ones_d = nc.dram_tensor("ones", [1, 128], F32, kind="ExternalInput").ap()
    onesv_d = nc.dram_tensor("onesv", [128, NST * HPC], F32,
                             kind="ExternalInput").ap()
    out_d = nc.dram_tensor("out", [S, E], BF16, kind="ExternalOutput").ap()

    ctxpool = tc.tile_pool

    with ctxpool(name="persist", bufs=1) as pp:
        # ---- persistent SBUF tensors ----
        xt_sb = pp.tile([128, NEC * S], BF16)       # X^T, e-chunk ec at [ec*S)
        wqt_sb = pp.tile([128, NEC * C], BF16)
        wkt_sb = pp.tile([128, NEC * C], BF16)
        wvt_sb = pp.tile([128, NEC * C], BF16)
        wot_sb = pp.tile([128, 2 * E], BF16)        # c-chunk cc at [cc*E)
        qt_sb = pp.tile([128, 2 * S], BF16)         # Q^T, d-tile t at [t*S)
        kt_sb = pp.tile([128, 2 * S], BF16)
        v_sb = pp.tile([128, NST * VW], BF16)       # V (+ones col per head)
        ot_sb = pp.tile([128, 2 * S], BF16)         # normalized attn out^T
        bqk_sb = pp.tile([128, 4], F32)
        bvb_sb = pp.tile([128, C], F32)             # bv broadcast to partitions
        ones_r = pp.tile([1, 128], F32R)
        bv_row = pp.tile([1, C], F32R)
        onesb_sb = pp.tile([128, NST * HPC], F32)

        # ---- DMA: two queues in consumption order.
        # scalar queue: tiny tensors, wq, wv (frees up before first exp)
        # sync queue: x chunks + wk, then wot, then x for qb1..3 ----
        # Input stream in strict consumption order.  DIRECT2D bandwidth is a
        # shared ~200GB/s regardless of queue, so the head loads ONLY what
        # the first window needs: x-qb0 + the dt0 halves of wq/wk (1.5MB),
        # then wv, then the dt1 halves, then the rest.  Tiny tensors ride
        # the scalar queue interleaved so no whole-tensor record gates PE.
        # DMA bandwidth is ~200GB/s GLOBAL (shared by all queues), so strict
        # phase order matters: the 1.5MB the first window needs goes first
        # on every queue, then wv, then the dt1 weight halves, then the rest.
        smalls = [(bqk_sb, bqk_d, F32), (ones_r, ones_d, F32R),
                  (bv_row, bv_d, F32R), (onesb_sb, onesv_d, F32)]
        # phase 1: x-qb0 + dt0 halves of wq/wk
        for ec in range(NEC):
            nc.scalar.dma_start(
                out=wqt_sb[:, ec * C: ec * C + 128],
                in_=wqt_d[ec * 128:(ec + 1) * 128, 0:128])
            if smalls:
                dst, src, dt_ = smalls.pop(0)
                nc.scalar.dma_start(
                    out=dst[:], in_=src[:] if dt_ != F32R else src[:].bitcast(F32R))
            q = nc.sync if ec % 2 == 0 else nc.gpsimd
            q.dma_start(out=xt_sb[:, ec * S: ec * S + 512],
                        in_=xt_d[ec * 128:(ec + 1) * 128, 0:512])
            nc.sync.dma_start(
                out=wkt_sb[:, ec * C: ec * C + 128],
                in_=wkt_d[ec * 128:(ec + 1) * 128, 0:128])
        # phase 2+: wv then wq-dt1 on gpsimd (done before affines begin);
        # wk-dt1, wot, x-qb2/3 on sync; x-qb1 on scalar (frees by ~15us so
        # the exp stream is never queued behind a DMA)
        for ec in range(NEC):
            nc.gpsimd.dma_start(out=wvt_sb[:, ec * C:(ec + 1) * C],
                                in_=wvt_d[ec * 128:(ec + 1) * 128, :])
        for ec in range(NEC):
            nc.gpsimd.dma_start(
                out=wqt_sb[:, ec * C + 128: (ec + 1) * C],
                in_=wqt_d[ec * 128:(ec + 1) * 128, 128:C])
            nc.sync.dma_start(
                out=wkt_sb[:, ec * C + 128: (ec + 1) * C],
                in_=wkt_d[ec * 128:(ec + 1) * 128, 128:C])
            nc.scalar.dma_start(
                out=xt_sb[:, ec * S + 512: ec * S + 1024],
                in_=xt_d[ec * 128:(ec + 1) * 128, 512:1024])
        for cc in range(2):
            nc.sync.dma_start(out=wot_sb[:, cc * E:(cc + 1) * E],
                              in_=wot_d[cc * 128:(cc + 1) * 128, :])
        for ec in range(NEC):
            nc.sync.dma_start(
                out=xt_sb[:, ec * S + 1024: (ec + 1) * S],
                in_=xt_d[ec * 128:(ec + 1) * 128, 1024:S])

        # V ones columns via a strided DVE copy
        v_ones_ap = v_sb[:].rearrange("p (n x) -> p n x", x=DH + 1)[:, :, DH:DH + 1]
        nc.vector.tensor_copy(
            v_ones_ap, onesb_sb[:].rearrange("p (n x) -> p n x", x=1))

        with ctxpool(name="proj_ps", bufs=2, space="PSUM") as proj_ps, \
             ctxpool(name="score_ps", bufs=2, space="PSUM") as score_ps, \
             ctxpool(name="attn_ps", bufs=2, space="PSUM") as attn_ps, \
             ctxpool(name="pt_pool", bufs=4) as pt_pool, \
             ctxpool(name="rs_pool", bufs=4) as rs_pool, \
             ctxpool(name="bc_pool", bufs=4) as bc_pool, \
             ctxpool(name="out_pool", bufs=4) as out_pool:

            # ---------------- filler micro-steps ----------------
            def qkproj_steps(qb, dt, pj):
                w_sb = wqt_sb if pj == 0 else wkt_sb
                o_sb = qt_sb if pj == 0 else kt_sb
                st8 = {}

                def mk(e0):
                    def f():
                        if e0 == 0:
                            st8['ps'] = proj_ps.tile(
                                [128, 512], F32, tag="pp",
                                name=f"qk{qb}{dt}{pj}")
                        for ec in range(e0, e0 + 4):
                            nc.tensor.matmul(
                                st8['ps'][:],
                                w_sb[:, ec * C + dt * 128:
                                     ec * C + dt * 128 + 128],
                                xt_sb[:, ec * S + qb * 512:
                                      ec * S + qb * 512 + 512],
                                start=(ec == 0), stop=(ec == NEC - 1),
                                skip_group_check=True)
                        if e0 + 4 == NEC:
                            nc.vector.tensor_scalar_add(
                                o_sb[:, dt * S + qb * 512:
                                     dt * S + qb * 512 + 512],
                                st8['ps'][:],
                                bqk_sb[:, 2 * pj + dt: 2 * pj + dt + 1])
                    return (4 * 512 * PE_NS, f)
                return [mk(e0) for e0 in (0, 4)]

            def vproj_steps(st):
                st8 = {}

                def mk(e0):
                    def f():
                        if e0 == 0:
                            st8['ps'] = proj_ps.tile([128, C], F32, tag="pp",
                                                     name=f"vp{st}")
                        for ec in range(e0, e0 + 4):
                            nc.tensor.matmul(
                                st8['ps'][:],
                                xt_sb[:, ec * S + st * 128:
                                      ec * S + st * 128 + 128],
                                wvt_sb[:, ec * C: (ec + 1) * C],
                                start=(ec == 0), stop=(ec == NEC - 1),
                                skip_group_check=True)
                        if e0 + 4 == NEC:
                            dst = v_sb[:, st * VW: st * VW + VW].rearrange(
                                "p (h x) -> p h x", h=HPC)[:, :, 0:DH]
                            nc.vector.tensor_add(
                                dst,
                                st8['ps'][:].rearrange("p (h x) -> p h x",
                                                       h=HPC),
                                bvb_sb[:].rearrange("p (h x) -> p h x", h=HPC))
                    return (4 * 256 * PE_NS, f)
                return [mk(e0) for e0 in (0, 4)]

            def outproj_steps(st, tail=False):
                # atomic: the psum tile shares rotation with score tiles, so
                # all its writers+readers must be emitted contiguously
                def f():
                    ps = score_ps.tile([128, 1024], F32, tag="sc",
                                       name=f"op{st}")
                    for eb in range(2):
                        for cc in range(2):
                            nc.tensor.matmul(
                                ps[:, eb * 512:(eb + 1) * 512],
                                ot_sb[:, cc * S + st * 128:
                                      cc * S + st * 128 + 128],
                                wot_sb[:, cc * E + eb * 512:
                                       cc * E + eb * 512 + 512],
                                start=(cc == 0), stop=(cc == 1),
                                skip_group_check=True)
                    o_t = out_pool.tile([128, E], BF16, tag="ob",
                                        name=f"ot{st}")
                    nc.vector.tensor_copy(o_t[:, 0:512], ps[:, 0:512])
                    if tail:
                        # ACT/gpsimd are idle once the last exp retires;
                        # split work so the drain overlaps across queues
                        nc.scalar.copy(o_t[:, 512:1024], ps[:, 512:1024])
                        for h, q in ((0, nc.sync), (1, nc.gpsimd)):
                            q.dma_start(
                                out=out_d[st * 128:(st + 1) * 128,
                                          h * 512:(h + 1) * 512],
                                in_=o_t[:, h * 512:(h + 1) * 512])
                    else:
                        # mid-window: stay off the ACT/gpsimd queues (exp
                        # and affine_select must not be delayed)
                        nc.vector.tensor_copy(o_t[:, 512:1024],
                                              ps[:, 512:1024])
                        nc.sync.dma_start(
                            out=out_d[st * 128:(st + 1) * 128, :], in_=o_t[:])
                return [(4 * 512 * PE_NS, f)]

            # filler queue: (pe_ns, key, fn); key = win*100 + kt of first use
            fillers = []
            for qb in range(NQB):
                for dt in range(2):
                    if qb == 0 and dt == 0:
                        continue   # emitted directly in the head phase
                    for pj in range(2):
                        for s in qkproj_steps(qb, dt, pj):
                            fillers.append((s[0], (2 * qb + dt) * 100, s[1]))
            for st in range(0, NST):
                key = (2 * (st // 4) * 100 + st) if causal else st
                for s in vproj_steps(st):
                    fillers.append((s[0], key, s[1]))
            fillers.sort(key=lambda f: f[1])

            # per-window ACT / attention-PE ns and filler quotas (static)
            def w0_of(qb, kt):
                j = kt - 4 * qb
                return 128 * j if (causal and j > 0) else 0

            win_act, win_pe = [], []
            for qb in range(NQB):
                nk = 4 * (qb + 1) if causal else NST
                for hp in range(2):
                    a = p = 0.0
                    for kt in range(nk):
                        w = 512 - w0_of(qb, kt)
                        a += 2 * w * ACT_NS + 190
                        p += 3 * w * PE_NS
                    win_act.append(a)
                    win_pe.append(p)
            total_filler = sum(f[0] for f in fillers) + 16 * 4 * 512 * PE_NS
            deficits = [max(0.0, a - p) for a, p in zip(win_act, win_pe)]
            spare = max(0.0, total_filler - sum(deficits))
            quota = [d + spare / len(deficits) for d in deficits]

            mtr = {"popped": 0.0, "target": 0.0}

            def pop_fillers(force_key=None, pace=None):
                while fillers:
                    pe_ns, key, fn = fillers[0]
                    if force_key is not None and key <= force_key:
                        pass
                    elif pace is not None and mtr["popped"] < pace:
                        pass
                    else:
                        break
                    fillers.pop(0)
                    fn()
                    mtr["popped"] += pe_ns

            # ---------------- head phase: qb0/dt0 projections ----------------
            for pj in range(2):
                for _, f in qkproj_steps(0, 0, pj):
                    f()
            # bv broadcast to all partitions via K=1 matmul (deps are tiny
            # DMAs done long ago; placed here so it can't block the PE head)
            ps_bv = proj_ps.tile([128, C], F32, tag="pp", name="bvb")
            nc.tensor.matmul(ps_bv[:], ones_r[0:1, 0:128], bv_row[:],
                             start=True, stop=True)
            nc.vector.tensor_copy(bvb_sb[:], ps_bv[:])

            # ---------------- attention + out-proj ----------------
            pending = []

            def flush_pending():
                while pending:
                    pending.pop(0)()

            for qb in range(NQB):
                nk = 4 * (qb + 1) if causal else NST
                q0 = qb * 512
                for hp in range(2):
                    win = 2 * qb + hp
                    pop_fillers(force_key=win * 100)
                    win_base = mtr["popped"]
                    t = hp
                    ps_os = [None, None]

                    def emit_pv(kt, pt, w0, hp=hp, nk=nk, ps_os=ps_os, qb=qb):
                        if kt == 0:
                            for a in range(2):
                                ps_os[a] = attn_ps.tile(
                                    [65, 512], F32, tag="po",
                                    name=f"po{qb}{hp}{a}")
                        for a in range(2):
                            h = 2 * hp + a
                            nc.tensor.matmul(
                                ps_os[a][:, w0:512],
                                v_sb[:, kt * VW + h * (DH + 1):
                                     kt * VW + h * (DH + 1) + DH + 1],
                                pt[:, a * 512 + w0:(a + 1) * 512],
                                start=(kt == 0), stop=(kt == nk - 1),
                                skip_group_check=True)


                    pv_queue = []
                    for kt in range(nk):
                        j = kt - 4 * qb
                        w0 = 128 * j if (causal and j > 0) else 0
                        ps_s = score_ps.tile([128, 1024], F32, tag="sc",
                                             name=f"sc{qb}{hp}{kt}")
                        pt = pt_pool.tile([128, 1024], BF16, tag="pt",
                                          name=f"pt{qb}{hp}{kt}")
                        for a in range(2):
                            p0 = a * 64
                            nc.tensor.matmul(
                                ps_s[:, a * 512 + w0:(a + 1) * 512],
                                kt_sb[p0:p0 + 64,
                                      t * S + kt * 128: t * S + kt * 128 + 128],
                                qt_sb[p0:p0 + 64,
                                      t * S + q0 + w0: t * S + q0 + 512],
                                start=True, stop=True)
                        ps3 = ps_s[:].rearrange("p (u q) -> p u q", u=2)
                        pt3 = pt[:].rearrange("p (u q) -> p u q", u=2)
                        nc.scalar.activation(pt3[:, :, w0:512],
                                             ps3[:, :, w0:512],
                                             ActF.Exp, scale=SCALE)
                        if causal and 0 <= j:
                            nc.gpsimd.affine_select(
                                out=pt3[:, :, w0:w0 + 128],
                                in_=pt3[:, :, w0:w0 + 128],
                                compare_op=Alu.is_ge,
                                fill=0.0, base=0,
                                pattern=[[0, 2], [1, 128]],
                                channel_multiplier=-1)
                        if kt == min(1, nk - 1):
                            flush_pending()
                        pop_fillers(
                            force_key=win * 100 + kt,
                            pace=win_base + quota[win] * (kt + 1) / nk)
                        pv_queue.append((kt, pt, w0))
                        if len(pv_queue) > 2:
                            emit_pv(*pv_queue.pop(0))
                    while pv_queue:
                        emit_pv(*pv_queue.pop(0))

                    def norm(qb=qb, hp=hp, t=t, q0=q0, ps_os=ps_os):
                        rs = [None, None]
                        ps_b = [None, None]
                        bc = [None, None]
                        for a in range(2):
                            rs[a] = rs_pool.tile([1, 512], F32R, tag="rs",
                                                 name=f"rs{qb}{hp}{a}")
                            nc.vector.tensor_copy(rs[a][:],
                                                  ps_os[a][64:65, :])
                        for a in range(2):
                            ps_b[a] = score_ps.tile([64, 512], F32, tag="sc",
                                                    name=f"pb{qb}{hp}{a}")
                            nc.tensor.matmul(ps_b[a][:], ones_r[0:1, 0:64],
                                             rs[a][:], start=True, stop=True)
                        for a in range(2):
                            bc[a] = bc_pool.tile([64, 512], F32, tag="bc",
                                                 name=f"bc{qb}{hp}{a}")
                            nc.vector.reciprocal_approx_fast(bc[a][:],
                                                             ps_b[a][:])
                        for a in range(2):
                            p0 = a * 64
                            nc.vector.tensor_mul(
                                ot_sb[p0:p0 + 64,
                                      t * S + q0: t * S + q0 + 512],
                                ps_os[a][0:64, :], bc[a][:])
                        if hp == 1:
                            tail = qb == NQB - 1
                            for i, st in enumerate(range(qb * 4, qb * 4 + 4)):
                                # spread-force into window 2qb+4 (tail for qb3)
                                key = 999999 if tail else \
                                    (2 * qb + 4) * 100 + 1 + 3 * i
                                for s in outproj_steps(st, tail=tail):
                                    fillers.append((s[0], key, s[1]))
                            fillers.sort(key=lambda f: f[1])
                    pending.append(norm)
            flush_pending()
            pop_fillers(force_key=999999)


def _build(causal):
    nc = bacc.Bacc("TRN2", target_bir_lowering=False, debug=False,
                   num_devices=NCORES)
    with tile.TileContext(nc) as tc:
        _emit(nc, tc, causal)
    nc.compile()
    return nc


def _shard_inputs(QKV, Wq, bq, Wk, bk, Wv, bv, Wo, bo):
    QKV = np.asarray(QKV, dtype=np.float32)
    Wq, Wk, Wv, Wo = (np.asarray(w, dtype=np.float32) for w in (Wq, Wk, Wv, Wo))
    bq, bk, bv = (np.asarray(b_, dtype=np.float32) for b_ in (bq, bk, bv))
    ones = np.ones((1, 128), dtype=np.float32)
    onesv = np.ones((128, NST * HPC), dtype=np.float32)
    bf = ml_dtypes.bfloat16
    in_maps = []
    for core in range(NCORES):
        b, g = divmod(core, TPW)
        cs = slice(g * C, (g + 1) * C)
        bqs, bks = bq[cs], bk[cs]
        bqk = np.stack([bqs[:128], bqs[128:], bks[:128], bks[128:]], axis=1)
        in_maps.append({
            "xt": np.ascontiguousarray(QKV[b].T).astype(bf),
            "wqt": np.ascontiguousarray(Wq[cs, :].T).astype(bf),
            "wkt": np.ascontiguousarray(Wk[cs, :].T).astype(bf),
            "wvt": np.ascontiguousarray(Wv[cs, :].T).astype(bf),
            "wot": np.ascontiguousarray(Wo[:, cs].T).astype(bf),
            "bqk": np.ascontiguousarray(bqk),
            "bv": bv[cs].reshape(1, C).copy(),
            "ones": ones,
            "onesv": onesv,
        })
    return in_maps


def kernel(QKV, Wq, bq, Wk, bk, Wv, bv, Wo, bo, is_causal):
    causal = bool(int(np.asarray(is_causal)))
    if causal not in _cache:
        _cache[causal] = _build(causal)
    nc = _cache[causal]
    in_maps = _shard_inputs(QKV, Wq, bq, Wk, bk, Wv, bv, Wo, bo)
    res = run_bass_kernel_spmd(nc, in_maps, core_ids=list(range(NCORES)))
    bo_f = np.asarray(bo, dtype=np.float32)
    out = np.empty((B, S, E), dtype=np.float32)
    for b in range(B):
        acc = res.results[TPW * b]["out"].astype(np.float32)
        for g in range(1, TPW):
            acc = acc + res.results[TPW * b + g]["out"].astype(np.float32)
        out[b] = acc + bo_f
    return out
